# revision 2
# baseline (speedup 1.0000x reference)
"""Multi-head cross-attention (post-LN) Trainium2 Bass kernel, v3 (fp8).

Sharding: 8 cores = 4 batches x 2 head-groups (8 heads each). Per core:
project Q/K/V for its 512 feature columns over full i=1024 / j=2048, scores
+ softmax + AV per head, out-projection partials for all 1024 rows, pairwise
ReduceScatter, post-LN on own 512 rows.

v3 changes vs v2 (bf16 baseline):
- All projection / AV / out-proj matmuls run fp8e4m3 with DoubleRow perf
  mode (2 contraction slices per instr). Weights are pre-scaled x16 on the
  host so fp8's 240-max range is well used; the x256 net score scale is
  folded into the exp constants, the x256 out-proj scale into the pb copy.
- Scores fp8 DoubleRow too: d_head 64 = 2x32 via DMA-repacked qP/kP
  [32, 2, *] operand layouts (SBUF->SBUF partition-split DMAs).
- The exp stream (the ACT bottleneck, 128 x [128,1024] tiles/rep) is split
  per-head across engines: ACT heads use native Exp -> fp8 (bias -4 keeps
  e^z under fp8 max 240); DVE heads use a uint8 bit-trick: fp8e4m3 bits =
  round(8*log2(p)) + 56, i.e. one tensor_scalar (z*11.54*S + 120.5) -> u8,
  bitcast to fp8. Per-(i,head) softmax scale factors cancel in the
  denominator, so each head just needs one consistent engine/scale.
- Residual is injected on the PE: po += ident.T @ (128*h) inside the
  out-proj PSUM group, so ReduceScatter output y already holds h + attn.
- Post-LN on bf16 (2-byte DVE modes), rstd = Newton rsqrt on DVE bit ops:
  no Ln/Exp ping-pong -> the only ACT table used all kernel is Exp's.
- K/V/avT/pb copies can run on the otherwise-idle Pool (gpsimd) engine.
"""

import sys

for _p in ("/opt/trn_rl_repo", "/root/.axon_site/_ro/trn_rl_repo"):
    if _p not in sys.path:
        sys.path.append(_p)

import numpy as np

import concourse.bass as bass
import concourse.tile as tile
from concourse import bacc, mybir
from concourse import masks

P = 128
D = 1024          # d_model
I = 1024          # query rows (full q_len)
J = 2048          # kv length
NHC = 8           # heads per core
DH = 64           # head dim
FT = 4            # feature tiles (512 per core / 128)
DT = 8            # d_model contraction tiles
JT = 16           # j tiles
SCALE = 1.0 / (DH ** 0.5)
LN_EPS = 1e-5
F32 = mybir.dt.float32
BF16 = mybir.dt.bfloat16
FP8 = mybir.dt.float8e4
FP8E5 = mybir.dt.float8e5
U8 = mybir.dt.uint8
U32 = mybir.dt.uint32
DRM = mybir.MatmulPerfMode.DoubleRow
GROUPS = [[0, 1], [2, 3], [4, 5], [6, 7]]

WS = 16.0                  # host-side weight prescale
SSC = SCALE / (WS * WS)    # score scale back to z units
# DVE bit-trick exp targets fp8e5m2: bits = 4*log2(p) + 62, covers
# z*log2e in [-15,+15] octaves (e4m3's 15-octave span is too narrow)
A_DVE = 4.0 * 1.4426950408889634 * SSC
B_DVE = 60.5
ACT_BIAS = -4.0

# per-head exp engine: "A" = ACT native exp, "D" = DVE u8 bit-trick
EXP_ENG = ["A", "D", "A", "D", "A", "D", "S", "S"]
# GPSIMD cannot touch PSUM, so PSUM->SBUF copies stay on DVE/ACT.
ACT_KCOPY = True    # k proj copies on ACT (Copy shares Exp's table)
POOL_LN = False     # Pool normalize: suspect AP-scalar TS on gpsimd
ACT_AVT = True      # avT copies on ACT (PSUM fp8 -> SBUF fp8)
IDENT_HRES = True   # residual via identity matmul into out-proj PSUM


def build_program(reps=1, debug=False):
    nc = bacc.Bacc(None, target_bir_lowering=False, debug=False, num_devices=8)

    hT = nc.dram_tensor("hT", [D, I], FP8, kind="ExternalInput")
    cT = nc.dram_tensor("cT", [D, J], FP8, kind="ExternalInput")
    wq = nc.dram_tensor("wq", [D, FT * P], FP8, kind="ExternalInput")
    wk = nc.dram_tensor("wk", [D, FT * P], FP8, kind="ExternalInput")
    wv = nc.dram_tensor("wv", [D, FT * P], FP8, kind="ExternalInput")
    wo = nc.dram_tensor("wo", [FT * P, D], FP8, kind="ExternalInput")
    hres = nc.dram_tensor("hres", [I, D], BF16, kind="ExternalInput")  # x128
    gamma = nc.dram_tensor("gamma", [P, D], BF16, kind="ExternalInput")
    beta = nc.dram_tensor("beta", [P, D], BF16, kind="ExternalInput")
    out = nc.dram_tensor("out", [I // 2, D], F32, kind="ExternalOutput")
    dbg = {}
    if debug:
        for nm, shape, dt_ in [
            ("d_qT", [P, FT, I], FP8), ("d_kT", [P, FT, J], FP8),
            ("d_qP", [32, NHC, 2, I], FP8), ("d_kP", [32, NHC, 2, J], FP8),
            ("d_v", [P, JT, NHC, 66], FP8), ("d_pT", [P, 8, I], U8),
            ("d_avn", [P, FT, 8, P], FP8), ("d_avT", [P, FT, I], FP8),
            ("d_y", [P, 4, D], BF16), ("d_mv", [P, 4, 2], F32),
            ("d_rstd", [P, 4], F32),
        ]:
            dbg[nm] = nc.dram_tensor(nm, shape, dt_, kind="ExternalOutput")

    with tile.TileContext(nc) as tc:
        with (
            tc.tile_pool(name="consts", bufs=1) as consts,
            tc.tile_pool(name="persist", bufs=1) as persist,
            tc.tile_pool(name="psum", bufs=1, space="PSUM") as psum,
            tc.tile_pool(name="dram", bufs=1, space="DRAM") as dram,
        ):
            gamma_bc = consts.tile([P, D], BF16, tag="gamma_bc")
            beta_bc = consts.tile([P, D], BF16, tag="beta_bc")
            ident8 = consts.tile([P, P], FP8, tag="ident8")
            masks.make_identity(nc, ident8[:])
            ident16 = consts.tile([P, P], BF16, tag="ident16")
            masks.make_identity(nc, ident16[:])
            abias_t = consts.tile([P, 1], F32, tag="abias")
            nc.vector.memset(abias_t, ACT_BIAS)

            cT_sb = persist.tile([P, DT, J], FP8, tag="cT")        # 16KB
            hT_sb = persist.tile([P, DT, 2, 512], FP8, tag="hT")   # 8KB
            wq_sb = persist.tile([P, DT, FT * P], FP8, tag="wq")   # 4KB
            wk_sb = persist.tile([P, DT, FT * P], FP8, tag="wk")   # 4KB
            wv_sb = persist.tile([P, DT, FT * P], FP8, tag="wv")   # 4KB
            wo_sb = persist.tile([P, FT, D], FP8, tag="wo")        # 4KB
            qT_sb = persist.tile([P, FT, I], FP8, tag="qT")        # 4KB
            kT_sb = persist.tile([P, FT, J], FP8, tag="kT")        # 8KB
            qP_sb = persist.tile([32, NHC, 2, I], FP8, tag="qP")   # 16KB
            kP_sb = persist.tile([32, NHC, 2, J], FP8, tag="kP")   # 32KB
            v_sb = persist.tile([P, JT, NHC, 66], FP8, tag="v")    # 8.25KB
            pT_sb = persist.tile([P, 8, I], U8, tag="pT")          # 8KB
            av_n = persist.tile([P, FT, 8, P], FP8, tag="av_n")    # 4KB
            avT_loc = persist.tile([P, FT, I], FP8, tag="avT")     # 4KB
            hres_sb = persist.tile([P, 8, D], BF16, tag="hres")    # 16KB

            pT8 = pT_sb.bitcast(FP8)      # ACT heads write e4m3
            pT85 = pT_sb.bitcast(FP8E5)   # DVE heads write e5m2 bit-trick

            rs_in = dram.tile([2, 2, 4, P, D], BF16)
            rs_out = dram.tile([2, 2, 2, P, D], BF16)

            nc.vector.memset(v_sb[:, :, :, DH:DH + 1], 1.0)

            def make_post_tiles(rbuf, capture=None):
                # post-ReduceScatter: y already includes the residual
                # (IDENT_HRES) or needs hres added. bf16 pipeline; rstd via
                # Newton rsqrt on DVE bit ops (no ACT table swaps).
                ys, mv4 = [], None

                def stats_fn(ci, k2):
                    nonlocal mv4
                    k = 2 * ci + k2
                    y = persist.tile([P, D], BF16, tag="y", name="y", bufs=8)
                    nc.sync.dma_start(y, rs_out[rbuf, ci, k2])
                    if not IDENT_HRES:
                        nc.vector.tensor_tensor(
                            y, y, hres_sb[:, k, :], mybir.AluOpType.add)
                    if mv4 is None:
                        mv4 = persist.tile(
                            [P, 4, nc.vector.BN_AGGR_DIM], F32,
                            tag="mv4", name="mv4", bufs=2)
                    stats = persist.tile(
                        [P, 2, nc.vector.BN_STATS_DIM], F32,
                        tag="stats", name="stats", bufs=2)
                    yg = y.rearrange("p (g d) -> p g d", g=2)
                    for g in range(2):
                        nc.vector.bn_stats(stats[:, g, :], yg[:, g, :])
                    nc.vector.bn_aggr(mv4[:, k, :], stats)
                    ys.append((k, y))

                def rstd_fn():
                    # rstd4 = rsqrt(var+eps), one Newton step
                    rstd4 = persist.tile([P, 4], F32, tag="rstd4",
                                         name="rstd4", bufs=2)
                    ve = persist.tile([P, 4], F32, tag="ve", name="ve",
                                      bufs=2)
                    nc.vector.tensor_scalar(
                        ve, mv4[:, :, 1], LN_EPS, None,
                        op0=mybir.AluOpType.add)
                    y0 = persist.tile([P, 4], F32, tag="y0", name="y0",
                                      bufs=2)
                    y0u = y0.bitcast(U32)
                    nc.vector.tensor_scalar(
                        y0u, ve.bitcast(U32), 1, None,
                        op0=mybir.AluOpType.logical_shift_right)
                    # 0x5F3759DF - (bits>>1), done as (-1)*t + C in f32;
                    # the ~100-ulp float rounding is far below Newton's
                    # own error floor
                    nc.vector.tensor_scalar(
                        y0u, y0u, -1.0, 1597463008.0,
                        op0=mybir.AluOpType.mult,
                        op1=mybir.AluOpType.add)
                    t1 = persist.tile([P, 4], F32, tag="t1", name="t1",
                                      bufs=2)
                    nc.vector.tensor_tensor(t1, y0, y0,
                                            mybir.AluOpType.mult)
                    nc.vector.tensor_tensor(t1, t1, ve,
                                            mybir.AluOpType.mult)
                    nc.vector.tensor_scalar(
                        t1, t1, -0.5, 1.5,
                        op0=mybir.AluOpType.mult,
                        op1=mybir.AluOpType.add)
                    nc.vector.tensor_tensor(rstd4, y0, t1,
                                            mybir.AluOpType.mult)
                    return rstd4

                def norm_fn(k, y, rstd4):
                    eng = nc.gpsimd if POOL_LN else nc.vector
                    xb = persist.tile([P, D], BF16, tag="xb", name="xb",
                                      bufs=2)
                    eng.tensor_scalar(
                        xb, y, mv4[:, k, 0:1], rstd4[:, k:k + 1],
                        op0=mybir.AluOpType.subtract,
                        op1=mybir.AluOpType.mult)
                    eng.tensor_tensor(xb, xb, gamma_bc,
                                      mybir.AluOpType.mult)
                    xf = persist.tile([P, D], F32, tag="xf", name="xf",
                                      bufs=2)
                    eng.tensor_tensor(xf, xb, beta_bc,
                                      mybir.AluOpType.add)
                    nc.sync.dma_start(out.ap()[k * P:(k + 1) * P, :], xf)

                fns = [lambda ci=ci, k2=k2: stats_fn(ci, k2)
                       for ci in range(2) for k2 in range(2)]

                def tail():
                    rstd4 = rstd_fn()
                    for k, y in ys:
                        norm_fn(k, y, rstd4)
                    if capture is not None and debug:
                        capture(rstd4, ys)
                        if mv4 is not None:
                            nc.sync.dma_start(dbg["d_mv"].ap(), mv4)

                fns.append(tail)
                return fns

            pending = []
            for _rep in range(reps):
                rbuf = _rep % 2
                # ---- input DMAs: few big partition-folded transfers,
                # split across the SP and ACT HWDGE queues
                def fold(dram_t, rows, cols, c0=0):
                    # DRAM [rows*128, cols] slice -> AP [128, rows, cols]
                    return dram_t.ap()[0:rows * P, c0:c0 + cols].rearrange(
                        "(r p) c -> p r c", p=P)

                nc.scalar.dma_start(wq_sb, fold(wq, DT, 512))
                nc.sync.dma_start(wk_sb, fold(wk, DT, 512))
                nc.scalar.dma_start(
                    hT_sb, fold(hT, DT, I).rearrange(
                        "p r (h c) -> p r h c", h=2))
                nc.sync.dma_start(cT_sb[:, :, 0:1024], fold(cT, DT, 1024, 0))
                nc.scalar.dma_start(cT_sb[:, :, 1024:2048],
                                    fold(cT, DT, 1024, 1024))
                nc.sync.dma_start(wv_sb, fold(wv, DT, 512))
                nc.sync.dma_start(wo_sb, fold(wo, FT, D))
                nc.scalar.dma_start(hres_sb, fold(hres, 8, D))
                nc.sync.dma_start(gamma_bc, gamma.ap())
                nc.sync.dma_start(beta_bc, beta.ap())

                post_fns = []
                if len(pending) == 2:
                    post_fns = make_post_tiles(pending.pop(0) % 2)

                # ---- PE chunks (fp8 DoubleRow) ---------------------------
                pcount = [0]

                def proj_ps(shape):
                    t = psum.tile(shape, F32,
                                  tag=("pA", "pB")[pcount[0] % 2],
                                  name="proj_ps")
                    pcount[0] += 1
                    return t

                def q_chunk(ft, ih, cp=None):
                    # qT[f 128, i 512] for feature block ft, i-half ih
                    ps = proj_ps([P, 512])
                    for nn in range(4):
                        for dp in range(4):
                            nc.tensor.matmul(
                                ps[:, nn * 128:(nn + 1) * 128],
                                wq_sb[:, 2 * dp:2 * dp + 2,
                                      ft * P:(ft + 1) * P],
                                hT_sb[:, 2 * dp:2 * dp + 2, ih,
                                      nn * 128:(nn + 1) * 128],
                                start=(dp == 0), stop=(dp == 3),
                                perf_mode=DRM,
                            )
                    (cp or nc.vector.tensor_copy)(
                        qT_sb[:, ft, ih * 512:(ih + 1) * 512], ps)

                def k_chunk(ft, jc, cp=None):
                    ps = proj_ps([P, 512])
                    for nn in range(4):
                        for dp in range(4):
                            nc.tensor.matmul(
                                ps[:, nn * 128:(nn + 1) * 128],
                                wk_sb[:, 2 * dp:2 * dp + 2,
                                      ft * P:(ft + 1) * P],
                                cT_sb[:, 2 * dp:2 * dp + 2,
                                      jc * 512 + nn * 128:
                                      jc * 512 + (nn + 1) * 128],
                                start=(dp == 0), stop=(dp == 3),
                                perf_mode=DRM,
                            )
                    (cp or nc.vector.tensor_copy)(
                        kT_sb[:, ft, jc * 512:(jc + 1) * 512], ps)

                def v_chunk(jq, fp, cp=None):
                    # v[j, feat] for 4 j-tiles x one 128-feat (2-head) block
                    ps = proj_ps([P, 4, P])
                    for k in range(4):
                        jt = jq * 4 + k
                        for dp in range(4):
                            nc.tensor.matmul(
                                ps[:, k, :],
                                cT_sb[:, 2 * dp:2 * dp + 2,
                                      jt * P:(jt + 1) * P],
                                wv_sb[:, 2 * dp:2 * dp + 2,
                                      fp * P:(fp + 1) * P],
                                start=(dp == 0), stop=(dp == 3),
                                perf_mode=DRM,
                            )
                    js = slice(jq * 4, (jq + 1) * 4)
                    cp = cp or nc.vector.tensor_copy
                    cp(v_sb[:, js, 2 * fp, 0:DH], ps[:, :, 0:DH])
                    cp(v_sb[:, js, 2 * fp + 1, 0:DH], ps[:, :, DH:P])

                def repack(ft):
                    # qP/kP [32, 2, *] packed layouts for DoubleRow scores
                    for hh in range(2):
                        h = 2 * ft + hh
                        for s in range(2):
                            r0 = hh * DH + s * 32
                            nc.sync.dma_start(
                                qP_sb[:, h, s, :],
                                qT_sb[r0:r0 + 32, ft, :])
                            nc.scalar.dma_start(
                                kP_sb[:, h, s, :],
                                kT_sb[r0:r0 + 32, ft, :])

                def run_chunk(spec, h):
                    # PSUM->SBUF copies go on the engine NOT doing this
                    # head's exp stream
                    cp = (nc.scalar.copy if EXP_ENG[h] == "D"
                          else nc.vector.tensor_copy)
                    kind, a, b = spec
                    if kind == "q":
                        q_chunk(a, b, cp)
                    elif kind == "k":
                        k_chunk(a, b, cp)
                    elif kind == "r":
                        repack(a)
                    else:
                        v_chunk(a, b, cp)

                # leveled placement: produce ft-block inputs one head-pair
                # ahead; repack DMAs fire right after their last source
                # chunk. v(jq,fp) lands before head 2fp uses pair 2jq.
                slot_map = {
                    0: ("v", 0, 0), 2: ("k", 1, 0), 4: ("v", 1, 0),
                    6: ("k", 1, 1), 8: ("v", 2, 0), 10: ("k", 1, 2),
                    12: ("v", 3, 0), 14: ("k", 1, 3),
                    17: ("q", 1, 0), 19: ("q", 1, 1), 20: ("r", 1, 0),
                    22: ("v", 0, 1), 25: ("v", 1, 1), 28: ("v", 2, 1),
                    30: ("v", 3, 1),
                    34: ("k", 2, 0), 36: ("k", 2, 1), 38: ("k", 2, 2),
                    40: ("k", 2, 3), 42: ("q", 2, 0), 44: ("q", 2, 1),
                    45: ("r", 2, 0),
                    48: ("v", 0, 2), 52: ("v", 1, 2), 56: ("v", 2, 2),
                    60: ("v", 3, 2),
                    64: ("k", 3, 0), 66: ("k", 3, 1), 68: ("k", 3, 2),
                    70: ("k", 3, 3), 72: ("q", 3, 0), 74: ("q", 3, 1),
                    75: ("r", 3, 0),
                    78: ("v", 0, 3), 81: ("v", 1, 3), 84: ("v", 2, 3),
                    87: ("v", 3, 3),
                }

                # ---- lead-in: Q ft0 + K ft0 + repack ft0 -----------------
                q_chunk(0, 0)
                q_chunk(0, 1)
                for jc in range(4):
                    k_chunk(0, jc)
                repack(0)

                # ---- merged head loop ------------------------------------
                def emit_av(h, pr):
                    # av[i, 0:65] += pT_pair.T @ v_pair (DoubleRow)
                    a = (2 * pr) % 8
                    for it in range(8):
                        if EXP_ENG[h] == "A" or (EXP_ENG[h] == "S"
                                                 and it < 4):
                            pv = pT8
                        else:
                            pv = pT85
                        nc.tensor.matmul(
                            (av_A, av_B)[it // 4][:, it % 4, 0:DH + 1],
                            pv[:, a:a + 2, it * P:(it + 1) * P],
                            v_sb[:, 2 * pr:2 * pr + 2, h, 0:DH + 1],
                            start=(pr == 0 and it % 4 == 0),
                            stop=(pr == 7 and it % 4 == 3),
                            perf_mode=DRM,
                        )

                for h in range(NHC):
                    ft, hh = divmod(h, 2)
                    av_A = psum.tile([P, 4, P], F32, tag="avA", name="av_A")
                    av_B = psum.tile([P, 4, P], F32, tag="avB", name="av_B")
                    for jt in range(JT):
                        sc = psum.tile([P, I], F32,
                                       tag=("scA", "scB")[jt % 2], name="sc")
                        for nn in range(8):
                            nc.tensor.matmul(
                                sc[:, nn * 128:(nn + 1) * 128],
                                kP_sb[:, h, :, jt * P:(jt + 1) * P],
                                qP_sb[:, h, :, nn * 128:(nn + 1) * 128],
                                start=True, stop=True,
                                perf_mode=DRM,
                            )
                        sl = jt % 8
                        if EXP_ENG[h] == "A":
                            nc.scalar.activation(
                                pT8[:, sl, :], sc,
                                mybir.ActivationFunctionType.Exp,
                                scale=SSC, bias=abias_t,
                            )
                        elif EXP_ENG[h] == "D":
                            nc.vector.tensor_scalar(
                                pT_sb[:, sl, :], sc, A_DVE, B_DVE,
                                op0=mybir.AluOpType.mult,
                                op1=mybir.AluOpType.add,
                            )
                        else:  # split: ACT takes i 0:512, DVE i 512:1024
                            nc.scalar.activation(
                                pT8[:, sl, 0:512], sc[:, 0:512],
                                mybir.ActivationFunctionType.Exp,
                                scale=SSC, bias=abias_t,
                            )
                            nc.vector.tensor_scalar(
                                pT_sb[:, sl, 512:1024], sc[:, 512:1024],
                                A_DVE, B_DVE,
                                op0=mybir.AluOpType.mult,
                                op1=mybir.AluOpType.add,
                            )
                        spec = slot_map.get(16 * h + jt)
                        if spec is not None:
                            run_chunk(spec, h)
                        if jt % 2 == 1:
                            emit_av(h, jt // 2)

                    # normalize: av * (1/denom) -> av_n (fp8), one
                    # broadcast tensor_tensor per PSUM tile
                    for g in range(2):
                        avp = (av_A, av_B)[g]
                        rec4 = persist.tile([P, 4], F32, tag="rec4",
                                            name="rec4", bufs=2)
                        nc.vector.reciprocal(rec4, avp[:, :, DH])
                        nc.vector.tensor_tensor(
                            av_n[:, ft, 4 * g:4 * g + 4,
                                 hh * DH:(hh + 1) * DH],
                            avp[:, :, 0:DH],
                            rec4[:, :].to_broadcast([P, 4, DH]),
                            mybir.AluOpType.mult,
                        )

                    if h == 6:
                        for fn in post_fns:
                            fn()
                        post_fns = []

                    if hh == 1:
                        # transpose av[i,f] -> avT[f,i] for this ft.
                        # fp8 PE-transpose requires output element step 2.
                        for g2 in range(2):
                            tp = proj_ps([P, 4, P])
                            tpb = tp.bitcast(FP8)
                            for k in range(4):
                                it = g2 * 4 + k
                                nc.tensor.transpose(
                                    tpb[:, k, 0:2 * P:2],
                                    av_n[:, ft, it, :], ident8)
                            eng = nc.scalar.copy if ACT_AVT \
                                else nc.vector.tensor_copy
                            eng(
                                avT_loc[:, ft, g2 * 512:(g2 + 1) * 512]
                                .rearrange("p (a b) -> p a b", a=4),
                                tpb[:, :, 0:2 * P:2])

                # ---- out-proj (DoubleRow over ft pairs + residual ident)
                # + pairwise ReduceScatter in bf16
                pocount = [0]
                for ci, its in enumerate(((0, 4, 1, 5), (2, 6, 3, 7))):
                    for it in its:
                        for ih2 in range(2):
                            po = psum.tile([P, 512], F32,
                                           tag=("avA", "avB")[pocount[0] % 2],
                                           name="po")
                            pocount[0] += 1
                            # residual FIRST: start=True pending-zeroes the
                            # whole bank; a trailing start would re-mark
                            # earlier chunks' bytes and drop their sums
                            if IDENT_HRES:
                                nc.tensor.matmul(
                                    po, ident16,
                                    hres_sb[:, it,
                                            ih2 * 512:(ih2 + 1) * 512],
                                    start=True, stop=False,
                                    skip_group_check=True,
                                )
                            for nn in range(4):
                                for fp in range(2):
                                    last = (nn == 3 and fp == 1)
                                    nc.tensor.matmul(
                                        po[:, nn * 128:(nn + 1) * 128],
                                        avT_loc[:, 2 * fp:2 * fp + 2,
                                                it * P:(it + 1) * P],
                                        wo_sb[:, 2 * fp:2 * fp + 2,
                                              ih2 * 512 + nn * 128:
                                              ih2 * 512 + (nn + 1) * 128],
                                        start=(fp == 0 and not IDENT_HRES),
                                        stop=(last if IDENT_HRES
                                              else fp == 1),
                                        perf_mode=DRM,
                                        skip_group_check=IDENT_HRES,
                                    )
                            pb = persist.tile([P, 512], BF16, tag="pb",
                                              name="pb", bufs=4)
                            nc.vector.tensor_scalar(
                                pb, po, 1.0 / (WS * WS), None,
                                op0=mybir.AluOpType.mult)
                            slot = 2 * (it // 4) + (it % 4) - 2 * ci
                            nc.scalar.dma_start(
                                rs_in[rbuf, ci, slot, :,
                                      ih2 * 512:(ih2 + 1) * 512], pb)
                    nc.gpsimd.collective_compute(
                        "ReduceScatter",
                        mybir.AluOpType.add,
                        replica_groups=GROUPS,
                        ins=[rs_in[rbuf, ci].opt()],
                        outs=[rs_out[rbuf, ci].opt()],
                    )
                pending.append(_rep)

            last_post = [None, None]

            def capture(rstd4, ys):
                last_post[0] = rstd4
                last_post[1] = ys

            for r in pending:
                for fn in make_post_tiles(r % 2, capture):
                    fn()

            if debug and last_post[0] is not None:
                nc.sync.dma_start(dbg["d_qT"].ap(), qT_sb)
                nc.sync.dma_start(dbg["d_kT"].ap(), kT_sb)
                nc.sync.dma_start(dbg["d_qP"].ap(), qP_sb)
                nc.sync.dma_start(dbg["d_kP"].ap(), kP_sb)
                nc.sync.dma_start(dbg["d_v"].ap(), v_sb)
                nc.sync.dma_start(dbg["d_pT"].ap(), pT_sb)
                nc.sync.dma_start(dbg["d_avn"].ap(), av_n)
                nc.sync.dma_start(dbg["d_avT"].ap(), avT_loc)
                rstd4, ys = last_post
                nc.sync.dma_start(dbg["d_rstd"].ap(), rstd4)
                for k, y in ys:
                    nc.sync.dma_start(dbg["d_y"].ap()[:, k, :], y)

    nc.compile()
    return nc


_NC_CACHE = {}


def _get_program(reps=1):
    if reps not in _NC_CACHE:
        _NC_CACHE[reps] = build_program(reps)
    return _NC_CACHE[reps]


def _make_in_maps(h, c, Wq, Wkv, Wo, gamma, beta):
    import ml_dtypes
    bf16 = ml_dtypes.bfloat16
    fp8 = mybir.dt.np(FP8)

    h = np.asarray(h, dtype=np.float32)
    c = np.asarray(c, dtype=np.float32)
    Wq = np.asarray(Wq, dtype=np.float32)
    Wkv = np.asarray(Wkv, dtype=np.float32)
    Wo = np.asarray(Wo, dtype=np.float32)
    gamma = np.asarray(gamma, dtype=np.float32)
    beta = np.asarray(beta, dtype=np.float32)

    q_len, batch, d_model = h.shape
    assert (q_len, batch, d_model) == (I, 4, D)

    wqT = (Wq.T * WS).astype(fp8)
    wkT = (Wkv[:D].T * WS).astype(fp8)
    wvT = (Wkv[D:].T * WS).astype(fp8)
    woT = np.ascontiguousarray((Wo.T * WS).astype(fp8))
    gamma_b = np.ascontiguousarray(
        np.broadcast_to(gamma, (P, D)).astype(bf16))
    beta_b = np.ascontiguousarray(
        np.broadcast_to(beta, (P, D)).astype(bf16))

    in_maps = []
    for core in range(8):
        b, g = divmod(core, 2)
        f0, f1 = g * 512, (g + 1) * 512
        in_maps.append({
            "hT": np.ascontiguousarray(h[:, b, :].T.astype(fp8)),
            "cT": np.ascontiguousarray(c[:, b, :].T.astype(fp8)),
            "wq": np.ascontiguousarray(wqT[:, f0:f1]),
            "wk": np.ascontiguousarray(wkT[:, f0:f1]),
            "wv": np.ascontiguousarray(wvT[:, f0:f1]),
            "wo": np.ascontiguousarray(woT[f0:f1, :]),
            "hres": np.ascontiguousarray(
                (h[:, b, :] * (WS * WS / 2.0)).astype(bf16)),
            "gamma": gamma_b,
            "beta": beta_b,
        })
    return in_maps


_RUNNER = None


def kernel(h, c, Wq, Wkv, Wo, gamma, beta):
    """Full-input entry point; compiled executable cached across calls."""
    global _RUNNER
    in_maps = _make_in_maps(h, c, Wq, Wkv, Wo, gamma, beta)
    if _RUNNER is None:
        _RUNNER = _KernelRunner(_get_program())
    core_outs = _RUNNER.run(in_maps)

    out = np.empty((I, 4, D), dtype=np.float32)
    for core in range(8):
        b, g = divmod(core, 2)
        out[g * 512:(g + 1) * 512, b, :] = core_outs[core]["out"]
    return out


class _KernelRunner:
    """Persistent jitted SPMD executor."""

    def __init__(self, nc):
        import jax
        from jax.experimental.shard_map import shard_map
        from jax.sharding import Mesh, NamedSharding, PartitionSpec
        from concourse import bass2jax, mybir as _mybir

        bass2jax.install_neuronx_cc_hook()
        self._jax = jax
        partition_name = (nc.partition_id_tensor.name
                          if nc.partition_id_tensor else None)
        in_names, out_names, out_avals, zero_outs = [], [], [], []
        for alloc in nc.m.functions[0].allocations:
            if not isinstance(alloc, _mybir.MemoryLocationSet):
                continue
            name = alloc.memorylocations[0].name
            if alloc.kind == "ExternalInput":
                if name != partition_name:
                    in_names.append(name)
            elif alloc.kind == "ExternalOutput":
                shape = tuple(alloc.tensor_shape)
                dtype = _mybir.dt.np(alloc.dtype)
                out_names.append(name)
                out_avals.append(jax.core.ShapedArray(shape, dtype))
                zero_outs.append(np.zeros(shape, dtype))
        self._in_names, self._out_names = in_names, out_names
        self._out_avals, self._zero_outs = out_avals, zero_outs
        n_params = len(in_names)
        all_in = list(in_names) + list(out_names)
        if partition_name is not None:
            all_in.append(partition_name)

        def _body(*args):
            operands = list(args)
            if partition_name is not None:
                operands.append(bass2jax.partition_id_tensor())
            return tuple(bass2jax._bass_exec_p.bind(
                *operands, out_avals=tuple(out_avals),
                in_names=tuple(all_in), out_names=tuple(out_names),
                lowering_input_output_aliases=(),
                sim_require_finite=True, sim_require_nnan=True, nc=nc))

        donate = tuple(range(n_params, n_params + len(out_avals)))
        devices = jax.devices()[:8]
        mesh = Mesh(np.asarray(devices), ("core",))
        specs = (PartitionSpec("core"),)
        self._sharded = jax.jit(
            shard_map(_body, mesh=mesh,
                      in_specs=specs * (n_params + len(out_avals)),
                      out_specs=specs * len(out_avals), check_rep=False),
            donate_argnums=donate, keep_unused=True)
        self._sh = NamedSharding(mesh, PartitionSpec("core"))

    def run(self, in_maps):
        jax = self._jax
        dev_in = [jax.device_put(
            np.concatenate([np.asarray(in_maps[c][nm]) for c in range(8)],
                           axis=0), self._sh)
            for nm in self._in_names]
        zs = [jax.device_put(
            np.zeros((8 * z.shape[0], *z.shape[1:]), z.dtype), self._sh)
            for z in self._zero_outs]
        out_arrs = self._sharded(*dev_in, *zs)
        return [
            {name: np.asarray(out_arrs[i]).reshape(
                8, *self._out_avals[i].shape)[c]
             for i, name in enumerate(self._out_names)}
            for c in range(8)
        ]


def bench_paired(inputs, pairs=10, hi_reps=8):
    r_lo = _BenchRunner(inputs, reps=1)
    r_hi = _BenchRunner(inputs, reps=hi_reps)
    r_lo.run(); r_hi.run(); r_lo.run(); r_hi.run()
    diffs = []
    for _ in range(pairs):
        t_lo = r_lo.run()
        t_hi = r_hi.run()
        diffs.append((t_hi - t_lo) / (hi_reps - 1.0))
    diffs.sort()
    med = diffs[len(diffs) // 2]
    print(f"bench_paired: per-body diffs(us) = "
          f"{[f'{d*1e6:.0f}' for d in diffs]} -> median {med*1e6:.0f}us")
    return med * 1e9


class _BenchRunner:
    def __init__(self, inputs, reps):
        import jax
        from jax.experimental.shard_map import shard_map
        from jax.sharding import Mesh, NamedSharding, PartitionSpec
        from concourse import bass2jax, mybir as _mybir

        bass2jax.install_neuronx_cc_hook()
        nc = _get_program(reps)
        in_maps = _make_in_maps(**inputs)
        partition_name = (nc.partition_id_tensor.name
                          if nc.partition_id_tensor else None)
        in_names, out_names, out_avals, zero_outs = [], [], [], []
        for alloc in nc.m.functions[0].allocations:
            if not isinstance(alloc, _mybir.MemoryLocationSet):
                continue
            name = alloc.memorylocations[0].name
            if alloc.kind == "ExternalInput":
                if name != partition_name:
                    in_names.append(name)
            elif alloc.kind == "ExternalOutput":
                shape = tuple(alloc.tensor_shape)
                dtype = _mybir.dt.np(alloc.dtype)
                out_names.append(name)
                out_avals.append(jax.core.ShapedArray(shape, dtype))
                zero_outs.append(np.zeros(shape, dtype))
        n_params = len(in_names)
        all_in = list(in_names) + list(out_names)
        if partition_name is not None:
            all_in.append(partition_name)

        def _body(*args):
            operands = list(args)
            if partition_name is not None:
                operands.append(bass2jax.partition_id_tensor())
            return tuple(bass2jax._bass_exec_p.bind(
                *operands, out_avals=tuple(out_avals), in_names=tuple(all_in),
                out_names=tuple(out_names), lowering_input_output_aliases=(),
                sim_require_finite=True, sim_require_nnan=True, nc=nc))

        donate = tuple(range(n_params, n_params + len(out_avals)))
        devices = jax.devices()[:8]
        mesh = Mesh(np.asarray(devices), ("core",))
        specs = (PartitionSpec("core"),)
        self._sharded = jax.jit(
            shard_map(_body, mesh=mesh,
                      in_specs=specs * (n_params + len(out_avals)),
                      out_specs=specs * len(out_avals), check_rep=False),
            donate_argnums=donate, keep_unused=True)
        sh = NamedSharding(mesh, PartitionSpec("core"))
        self._dev_in = [jax.device_put(
            np.concatenate([np.asarray(in_maps[c][nm]) for c in range(8)],
                           axis=0), sh)
            for nm in in_names]
        self._zero_outs = zero_outs
        self._sh = sh
        self._jax = jax

    def run(self):
        import time
        jax = self._jax
        zs = [jax.device_put(
            np.zeros((8 * z.shape[0], *z.shape[1:]), z.dtype), self._sh)
            for z in self._zero_outs]
        jax.block_until_ready(zs)
        t0 = time.perf_counter()
        out = self._sharded(*self._dev_in, *zs)
        jax.block_until_ready(out)
        return time.perf_counter() - t0


# revision 3
# speedup vs baseline: 1.1884x; 1.1884x over previous
"""Multi-head cross-attention (post-LN) Trainium2 Bass kernel, v2.

Full inputs -> full outputs. Sharding: 8 cores = 4 batches x 2 head-groups
(8 heads each, tensor-parallel on n_head). Each core projects Q/K/V only for
its 512 feature columns over the FULL 1024 query rows, so the K/V projections
are not duplicated (the v1 row-split baseline recomputed them per core pair).
After attention a tiny per-ft AllToAll (256KB, core pairs) swaps transposed
attention-vector halves so each core runs the out-projection + residual + LN
for only its own 512 output rows. All matmul operands are bf16 (f32 PSUM).

Per-core pipeline, software-pipelined at j-tile-pair granularity:
  lead-in:  qT[f,i] = wq.T @ hT (ft0), kT[f,j] = wk.T @ cT (ft0)
  head loop (h = 0..7, ft = h//2):
    scores sT[j,i] = kT_h.T @ qT_h  (PSUM pair, 2 j-tiles)
    pT = exp(SCALE*sT)              (one ACT instr per pair, N=2048 -> bf16)
    avT[i,65] += pT_tile.T @ v_aug  (ones-column gives softmax denom free)
    filler: remaining Q/K ft chunks + V quads (v[j,f] = cT_tile.T @ wv)
            keep the PE busy under the ACT-bound exp stream
    end of head: normalize by 1/denom; after odd heads: PE-transpose
    av -> avT[f,i], DMA to DRAM, AllToAll with pair core (overlapped)
  tail:     out rows = LN(avT_loc.T @ wo + hres) for the 4 owned i-tiles
"""

import sys

for _p in ("/opt/trn_rl_repo", "/root/.axon_site/_ro/trn_rl_repo"):
    if _p not in sys.path:
        sys.path.append(_p)

import numpy as np

import concourse.bass as bass
import concourse.tile as tile
from concourse import bacc, mybir
from concourse import masks

P = 128
D = 1024          # d_model
I = 1024          # query rows per core (full q_len)
J = 2048          # kv length
NHC = 8           # heads per core
DH = 64           # head dim
FT = 4            # feature tiles (512 per core / 128)
DT = 8            # d_model contraction tiles
JT = 16           # j tiles
NPAIR = JT // 2   # score pairs per head
SCALE = 1.0 / (DH ** 0.5)
LN_EPS = 1e-5
F32 = mybir.dt.float32
BF16 = mybir.dt.bfloat16
GROUPS = [[0, 1], [2, 3], [4, 5], [6, 7]]


def build_program(reps=1):
    nc = bacc.Bacc(None, target_bir_lowering=False, debug=False, num_devices=8)

    hT = nc.dram_tensor("hT", [D, I], BF16, kind="ExternalInput")
    cT = nc.dram_tensor("cT", [D, J], BF16, kind="ExternalInput")
    wq = nc.dram_tensor("wq", [D, FT * P], BF16, kind="ExternalInput")
    wk = nc.dram_tensor("wk", [D, FT * P], BF16, kind="ExternalInput")
    wv = nc.dram_tensor("wv", [D, FT * P], BF16, kind="ExternalInput")
    wo = nc.dram_tensor("wo", [FT * P, D], BF16, kind="ExternalInput")
    hres = nc.dram_tensor("hres", [I // 2, D], F32, kind="ExternalInput")
    gamma = nc.dram_tensor("gamma", [P, D], F32, kind="ExternalInput")
    beta = nc.dram_tensor("beta", [P, D], F32, kind="ExternalInput")
    out = nc.dram_tensor("out", [I // 2, D], F32, kind="ExternalOutput")

    with tile.TileContext(nc) as tc:
        with (
            tc.tile_pool(name="consts", bufs=1) as consts,
            tc.tile_pool(name="persist", bufs=1) as persist,
            tc.tile_pool(name="psum", bufs=1, space="PSUM") as psum,
            tc.tile_pool(name="dram", bufs=1, space="DRAM") as dram,
        ):
            gamma_bc = consts.tile([P, D], F32, tag="gamma_bc")
            beta_bc = consts.tile([P, D], F32, tag="beta_bc")
            eps_t = consts.tile([P, 1], F32, tag="eps")
            nc.vector.memset(eps_t, LN_EPS)
            ident = consts.tile([P, P], BF16, tag="ident")
            masks.make_identity(nc, ident[:])

            cT_sb = persist.tile([P, DT, J], BF16, tag="cT")       # 32KB
            wk_sb = persist.tile([P, DT, FT * P], BF16, tag="wk")  # 8KB
            wv_sb = persist.tile([P, DT, FT * P], BF16, tag="wv")  # 8KB
            wo_sb = persist.tile([P, FT, D], BF16, tag="wo")       # 8KB
            qT_sb = persist.tile([P, FT, I], BF16, tag="qT")       # 8KB
            kT_sb = persist.tile([P, FT, J], BF16, tag="kT")       # 16KB
            v_sb = persist.tile([P, JT, NHC, 66], BF16, tag="v")   # 16.5KB
            av_n = persist.tile([P, FT, 8, P], BF16, tag="av_n")   # 8KB
            avT_loc = persist.tile([P, FT, I], BF16, tag="avT")    # 8KB
            hres_sb = persist.tile([P, 4, D], F32, tag="hres")     # 16KB

            rs_in = dram.tile([2, 2, 4, P, D], BF16)
            rs_out = dram.tile([2, 2, 2, P, D], BF16)

            hT_sb = persist.tile([P, DT, 2, 512], BF16, tag="hT")  # 16KB
            wq_sb = persist.tile([P, DT, FT * P], BF16, tag="wq")  # 8KB
            pT_sb = persist.tile([P, 4, I], BF16, tag="pT")        # 8KB
            nc.vector.memset(v_sb[:, :, :, DH:DH + 1], 1.0)

            def make_post_tiles(rbuf):
                # post-ReduceScatter: fetch summed rows, residual + LN, out.
                # Returned as per-tile closures, emitted spread across the
                # head boundaries two reps later so the DVE queue never
                # gets a large contiguous block ahead of chunk copies.
                def tile_fn(ci, k2):
                    k = 2 * ci + k2
                    y = persist.tile([P, D], BF16, tag="y", name="y",
                                     bufs=2)
                    nc.sync.dma_start(y, rs_out[rbuf, ci, k2])
                    x = persist.tile([P, D], F32, tag="x", name="x",
                                     bufs=2)
                    nc.vector.tensor_tensor(
                        x, y, hres_sb[:, k, :], mybir.AluOpType.add)
                    stats = persist.tile(
                        [P, 2, nc.vector.BN_STATS_DIM], F32,
                        tag="stats", name="stats", bufs=2)
                    xg = x.rearrange("p (g d) -> p g d", g=2)
                    for g in range(2):
                        nc.vector.bn_stats(stats[:, g, :], xg[:, g, :])
                    mv = persist.tile([P, nc.vector.BN_AGGR_DIM], F32,
                                      tag="mv", name="mv", bufs=2)
                    nc.vector.bn_aggr(mv, stats)
                    # rstd = exp(-0.5*ln(var+eps)): Ln and Exp live in the
                    # same ACT table set as the score exp stream, so no
                    # LoadActFuncSet swap mid-kernel (Sqrt would force two)
                    rstd = persist.tile([P, 1], F32, tag="rstd",
                                        name="rstd", bufs=2)
                    nc.scalar.activation(
                        rstd, mv[:, 1:2],
                        mybir.ActivationFunctionType.Ln, bias=eps_t)
                    nc.scalar.activation(
                        rstd, rstd,
                        mybir.ActivationFunctionType.Exp, scale=-0.5)
                    nc.vector.tensor_scalar(
                        x, x, mv[:, 0:1], rstd,
                        op0=mybir.AluOpType.subtract,
                        op1=mybir.AluOpType.mult,
                    )
                    nc.vector.tensor_tensor(x, x, gamma_bc,
                                            mybir.AluOpType.mult)
                    nc.vector.tensor_tensor(x, x, beta_bc,
                                            mybir.AluOpType.add)
                    nc.sync.dma_start(out.ap()[k * P:(k + 1) * P, :], x)
                return [lambda ci=ci, k2=k2: tile_fn(ci, k2)
                        for ci in range(2) for k2 in range(2)]

            pending = []
            for _rep in range(reps):
                rbuf = _rep % 2
                # ---- input DMAs, in consumption-priority order -----------
                for dt in range(DT):
                    nc.sync.dma_start(wq_sb[:, dt, :],
                                      wq.ap()[dt * P:(dt + 1) * P, :])
                    nc.sync.dma_start(hT_sb[:, dt, 0, :],
                                      hT.ap()[dt * P:(dt + 1) * P, 0:512])
                for dt in range(DT):
                    nc.sync.dma_start(wk_sb[:, dt, :],
                                      wk.ap()[dt * P:(dt + 1) * P, :])
                    nc.sync.dma_start(hT_sb[:, dt, 1, :],
                                      hT.ap()[dt * P:(dt + 1) * P, 512:1024])
                for jh in range(2):
                    for dt in range(DT):
                        nc.sync.dma_start(
                            cT_sb[:, dt, jh * 1024:(jh + 1) * 1024],
                            cT.ap()[dt * P:(dt + 1) * P,
                                    jh * 1024:(jh + 1) * 1024])
                for dt in range(DT):
                    nc.sync.dma_start(wv_sb[:, dt, :],
                                      wv.ap()[dt * P:(dt + 1) * P, :])
                for ft in range(FT):
                    nc.sync.dma_start(wo_sb[:, ft, :],
                                      wo.ap()[ft * P:(ft + 1) * P, :])
                for k in range(4):
                    nc.sync.dma_start(hres_sb[:, k, :],
                                      hres.ap()[k * P:(k + 1) * P, :])
                nc.sync.dma_start(gamma_bc, gamma.ap())
                nc.sync.dma_start(beta_bc, beta.ap())

                post_fns = []
                if len(pending) == 2:
                    post_fns = make_post_tiles(pending.pop(0) % 2)

                # ---- PE filler chunks (~4096 cyc each) -------------------
                pcount = [0]

                def proj_ps(shape):
                    t = psum.tile(shape, F32,
                                  tag=("pA", "pB")[pcount[0] % 2],
                                  name="proj_ps")
                    pcount[0] += 1
                    return t

                def q_chunk(ft, ih):
                    ps = proj_ps([P, 512])
                    for dt in range(DT):
                        nc.tensor.matmul(
                            ps,
                            wq_sb[:, dt, ft * P:(ft + 1) * P],
                            hT_sb[:, dt, ih, :],
                            start=(dt == 0), stop=(dt == DT - 1),
                        )
                    nc.vector.tensor_copy(
                        qT_sb[:, ft, ih * 512:(ih + 1) * 512], ps)

                def k_chunk(ft, jc):
                    ps = proj_ps([P, 512])
                    for dt in range(DT):
                        nc.tensor.matmul(
                            ps,
                            wk_sb[:, dt, ft * P:(ft + 1) * P],
                            cT_sb[:, dt, jc * 512:(jc + 1) * 512],
                            start=(dt == 0), stop=(dt == DT - 1),
                        )
                    nc.vector.tensor_copy(
                        kT_sb[:, ft, jc * 512:(jc + 1) * 512], ps)

                def v_chunk(jq, fp):
                    # v[j, feat] for 4 j-tiles x one 128-feat (2-head)
                    # block; heads 2*fp and 2*fp+1 live in this block.
                    ps = proj_ps([P, 4, P])
                    for k in range(4):
                        jt = jq * 4 + k
                        for dt in range(DT):
                            nc.tensor.matmul(
                                ps[:, k, :],
                                cT_sb[:, dt, jt * P:(jt + 1) * P],
                                wv_sb[:, dt, fp * P:(fp + 1) * P],
                                start=(dt == 0), stop=(dt == DT - 1),
                            )
                    js = slice(jq * 4, (jq + 1) * 4)
                    nc.vector.tensor_copy(v_sb[:, js, 2 * fp, 0:DH],
                                          ps[:, :, 0:DH])
                    nc.vector.tensor_copy(v_sb[:, js, 2 * fp + 1, 0:DH],
                                          ps[:, :, DH:P])

                def run_chunk(spec):
                    kind, a, b = spec
                    if kind == "q":
                        q_chunk(a, b)
                    elif kind == "k":
                        k_chunk(a, b)
                    else:
                        v_chunk(a, b)

                # leveled placement: one ~1.7us PE chunk per j-tile slot
                # (16 slots per head), each placed before its consumer's
                # deadline so the PE has work while ACT chews the exp stream
                slot_map = {
                    0: ("v", 0, 0), 2: ("k", 0, 1), 4: ("v", 1, 0),
                    6: ("k", 0, 2), 8: ("v", 2, 0), 10: ("k", 0, 3),
                    12: ("v", 3, 0),
                    18: ("k", 1, 0), 21: ("k", 1, 1), 24: ("k", 1, 2),
                    27: ("k", 1, 3), 29: ("q", 1, 0), 31: ("q", 1, 1),
                    32: ("v", 0, 1), 36: ("v", 1, 1), 40: ("v", 2, 1),
                    44: ("v", 3, 1),
                    50: ("k", 2, 0), 53: ("k", 2, 1), 56: ("k", 2, 2),
                    59: ("k", 2, 3), 61: ("q", 2, 0), 63: ("q", 2, 1),
                    64: ("v", 0, 2), 68: ("v", 1, 2), 72: ("v", 2, 2),
                    76: ("v", 3, 2),
                    82: ("k", 3, 0), 85: ("k", 3, 1), 88: ("k", 3, 2),
                    91: ("k", 3, 3), 93: ("q", 3, 0), 95: ("q", 3, 1),
                    96: ("v", 0, 3), 100: ("v", 1, 3), 104: ("v", 2, 3),
                    108: ("v", 3, 3),
                }

                # ---- lead-in: Q ft0 + first K ft0 chunk ------------------
                q_chunk(0, 0)
                q_chunk(0, 1)
                k_chunk(0, 0)

                # ---- merged head loop ------------------------------------
                def emit_av(h, jt):
                    # av[i, 0:64] += pT_tile.T @ v ; av[i, 64] += sum_j p
                    # PSUM start/stop is per 2KB bank (lazy zeroing): open
                    # each bank once; later accumulators' first writes hit
                    # pending-zero bytes and overwrite as needed.
                    for it in range(8):
                        nc.tensor.matmul(
                            (av_A, av_B)[it // 4][:, it % 4, 0:DH + 1],
                            pT_sb[:, jt % 4, it * P:(it + 1) * P],
                            v_sb[:, jt, h, 0:DH + 1],
                            start=(jt == 0 and it % 4 == 0),
                            stop=(jt == JT - 1 and it % 4 == 3),
                        )

                for h in range(NHC):
                    ft, hh = divmod(h, 2)
                    p0, p1 = hh * DH, (hh + 1) * DH
                    # bank-padded: each of the 4 accumulators owns a
                    # 512B quarter-bank slot (only 0:65 used)
                    av_A = psum.tile([P, 4, P], F32, tag="avA", name="av_A")
                    av_B = psum.tile([P, 4, P], F32, tag="avB", name="av_B")
                    for jt in range(JT):
                        sc = psum.tile([P, I], F32,
                                       tag=("scA", "scB")[jt % 2], name="sc")
                        for ih in range(2):
                            nc.tensor.matmul(
                                sc[:, ih * 512:(ih + 1) * 512],
                                kT_sb[p0:p1, ft, jt * P:(jt + 1) * P],
                                qT_sb[p0:p1, ft, ih * 512:(ih + 1) * 512],
                                start=True, stop=True,
                            )
                        nc.scalar.activation(
                            pT_sb[:, jt % 4, :], sc,
                            mybir.ActivationFunctionType.Exp,
                            scale=SCALE,
                        )
                        spec = slot_map.get(16 * h + jt)
                        if spec is not None:
                            run_chunk(spec)
                        if jt > 0:
                            emit_av(h, jt - 1)
                    emit_av(h, JT - 1)

                    # normalize: av / denom -> av_n (bf16)
                    for it in range(8):
                        avp = (av_A, av_B)[it // 4]
                        rec = persist.tile([P, 1], F32, tag="rec",
                                           name="rec", bufs=2)
                        nc.vector.reciprocal(rec, avp[:, it % 4, DH:DH + 1])
                        nc.vector.tensor_scalar(
                            av_n[:, ft, it, p0:p1],
                            avp[:, it % 4, 0:DH],
                            rec, None, op0=mybir.AluOpType.mult,
                        )

                    if h == 6:
                        # post-LN of the rep before last: head 6-7's DVE
                        # load is light (no filler copies) and the PE is
                        # exp-gated here, so this rides for free
                        for fn in post_fns:
                            fn()
                        post_fns = []

                    if hh == 1:
                        # transpose av[i,f] -> avT[f,i] for this ft
                        for g2 in range(2):
                            tp = proj_ps([P, 4, P])
                            tpb = tp.bitcast(BF16)
                            for k in range(4):
                                it = g2 * 4 + k
                                nc.tensor.transpose(
                                    tpb[:, k, 0:P],
                                    av_n[:, ft, it, :], ident)
                            nc.vector.tensor_copy(
                                avT_loc[:, ft, g2 * 512:(g2 + 1) * 512]
                                .rearrange("p (a b) -> p a b", a=4),
                                tpb[:, :, 0:P])

                # ---- out-proj (partials, all 1024 rows) + pairwise
                # ReduceScatter in bf16; the post-LN half is deferred
                pocount = [0]
                for ci, its in enumerate(((0, 4, 1, 5), (2, 6, 3, 7))):
                    for it in its:
                        for ih2 in range(2):
                            # reuse the av banks: frees pA/pB for the next
                            # rep's lead-in chunks to overlap this tail
                            po = psum.tile([P, 512], F32,
                                           tag=("avA", "avB")[pocount[0] % 2],
                                           name="po")
                            pocount[0] += 1
                            for ft in range(FT):
                                nc.tensor.matmul(
                                    po,
                                    avT_loc[:, ft, it * P:(it + 1) * P],
                                    wo_sb[:, ft, ih2 * 512:(ih2 + 1) * 512],
                                    start=(ft == 0), stop=(ft == FT - 1),
                                )
                            pb = persist.tile([P, 512], BF16, tag="pb",
                                              name="pb", bufs=4)
                            nc.vector.tensor_copy(pb, po)
                            # row slot within the chunk: own-half rows
                            # first, partner-half rows second
                            slot = 2 * (it // 4) + (it % 4) - 2 * ci
                            nc.scalar.dma_start(
                                rs_in[rbuf, ci, slot, :,
                                      ih2 * 512:(ih2 + 1) * 512], pb)
                    nc.gpsimd.collective_compute(
                        "ReduceScatter",
                        mybir.AluOpType.add,
                        replica_groups=GROUPS,
                        ins=[rs_in[rbuf, ci].opt()],
                        outs=[rs_out[rbuf, ci].opt()],
                    )
                pending.append(_rep)

            for r in pending:
                for fn in make_post_tiles(r % 2):
                    fn()

    nc.compile()
    return nc


_NC_CACHE = {}


def _get_program(reps=1):
    if reps not in _NC_CACHE:
        _NC_CACHE[reps] = build_program(reps)
    return _NC_CACHE[reps]


def _make_in_maps(h, c, Wq, Wkv, Wo, gamma, beta):
    import ml_dtypes
    bf16 = ml_dtypes.bfloat16

    h = np.asarray(h, dtype=np.float32)
    c = np.asarray(c, dtype=np.float32)
    Wq = np.asarray(Wq, dtype=np.float32)
    Wkv = np.asarray(Wkv, dtype=np.float32)
    Wo = np.asarray(Wo, dtype=np.float32)
    gamma = np.asarray(gamma, dtype=np.float32)
    beta = np.asarray(beta, dtype=np.float32)

    q_len, batch, d_model = h.shape
    assert (q_len, batch, d_model) == (I, 4, D)

    wqT = Wq.T.astype(bf16)
    wkT = Wkv[:D].T.astype(bf16)
    wvT = Wkv[D:].T.astype(bf16)
    woT = np.ascontiguousarray(Wo.T.astype(bf16))
    gamma_b = np.ascontiguousarray(np.broadcast_to(gamma, (P, D)))
    beta_b = np.ascontiguousarray(np.broadcast_to(beta, (P, D)))

    in_maps = []
    for core in range(8):
        b, g = divmod(core, 2)
        f0, f1 = g * 512, (g + 1) * 512
        in_maps.append({
            "hT": np.ascontiguousarray(h[:, b, :].T.astype(bf16)),
            "cT": np.ascontiguousarray(c[:, b, :].T.astype(bf16)),
            "wq": np.ascontiguousarray(wqT[:, f0:f1]),
            "wk": np.ascontiguousarray(wkT[:, f0:f1]),
            "wv": np.ascontiguousarray(wvT[:, f0:f1]),
            "wo": np.ascontiguousarray(woT[f0:f1, :]),
            "hres": np.ascontiguousarray(h[g * 512:(g + 1) * 512, b, :]),
            "gamma": gamma_b,
            "beta": beta_b,
        })
    return in_maps


_RUNNER = None


def kernel(h, c, Wq, Wkv, Wo, gamma, beta):
    """Full-input entry point. The compiled executable is cached across
    calls so repeat invocations only pay transfer + execute."""
    global _RUNNER
    in_maps = _make_in_maps(h, c, Wq, Wkv, Wo, gamma, beta)
    if _RUNNER is None:
        _RUNNER = _KernelRunner(_get_program())
    core_outs = _RUNNER.run(in_maps)

    out = np.empty((I, 4, D), dtype=np.float32)
    for core in range(8):
        b, g = divmod(core, 2)
        out[g * 512:(g + 1) * 512, b, :] = core_outs[core]["out"]
    return out


class _KernelRunner:
    """Persistent jitted SPMD executor (mirrors bass2jax.run_bass_via_pjrt,
    but reusable across calls with fresh inputs)."""

    def __init__(self, nc):
        import jax
        from jax.experimental.shard_map import shard_map
        from jax.sharding import Mesh, NamedSharding, PartitionSpec
        from concourse import bass2jax, mybir as _mybir

        bass2jax.install_neuronx_cc_hook()
        self._jax = jax
        partition_name = (nc.partition_id_tensor.name
                          if nc.partition_id_tensor else None)
        in_names, out_names, out_avals, zero_outs = [], [], [], []
        for alloc in nc.m.functions[0].allocations:
            if not isinstance(alloc, _mybir.MemoryLocationSet):
                continue
            name = alloc.memorylocations[0].name
            if alloc.kind == "ExternalInput":
                if name != partition_name:
                    in_names.append(name)
            elif alloc.kind == "ExternalOutput":
                shape = tuple(alloc.tensor_shape)
                dtype = _mybir.dt.np(alloc.dtype)
                out_names.append(name)
                out_avals.append(jax.core.ShapedArray(shape, dtype))
                zero_outs.append(np.zeros(shape, dtype))
        self._in_names, self._out_names = in_names, out_names
        self._out_avals, self._zero_outs = out_avals, zero_outs
        n_params = len(in_names)
        all_in = list(in_names) + list(out_names)
        if partition_name is not None:
            all_in.append(partition_name)

        def _body(*args):
            operands = list(args)
            if partition_name is not None:
                operands.append(bass2jax.partition_id_tensor())
            return tuple(bass2jax._bass_exec_p.bind(
                *operands, out_avals=tuple(out_avals),
                in_names=tuple(all_in), out_names=tuple(out_names),
                lowering_input_output_aliases=(),
                sim_require_finite=True, sim_require_nnan=True, nc=nc))

        donate = tuple(range(n_params, n_params + len(out_avals)))
        devices = jax.devices()[:8]
        mesh = Mesh(np.asarray(devices), ("core",))
        specs = (PartitionSpec("core"),)
        self._sharded = jax.jit(
            shard_map(_body, mesh=mesh,
                      in_specs=specs * (n_params + len(out_avals)),
                      out_specs=specs * len(out_avals), check_rep=False),
            donate_argnums=donate, keep_unused=True)
        self._sh = NamedSharding(mesh, PartitionSpec("core"))

    def run(self, in_maps):
        jax = self._jax
        dev_in = [jax.device_put(
            np.concatenate([np.asarray(in_maps[c][nm]) for c in range(8)],
                           axis=0), self._sh)
            for nm in self._in_names]
        zs = [jax.device_put(
            np.zeros((8 * z.shape[0], *z.shape[1:]), z.dtype), self._sh)
            for z in self._zero_outs]
        out_arrs = self._sharded(*dev_in, *zs)
        return [
            {name: np.asarray(out_arrs[i]).reshape(
                8, *self._out_avals[i].shape)[c]
             for i, name in enumerate(self._out_names)}
            for c in range(8)
        ]


def bench_paired(inputs, pairs=10, hi_reps=8):
    """Paired-difference timing: interleave isolated calls of the reps=1 and
    reps=hi NEFFs; median of (t_hi - t_lo)/(hi-1) cancels slow drift."""
    r_lo = _BenchRunner(inputs, reps=1)
    r_hi = _BenchRunner(inputs, reps=hi_reps)
    r_lo.run(); r_hi.run(); r_lo.run(); r_hi.run()  # warm both
    diffs = []
    for _ in range(pairs):
        t_lo = r_lo.run()
        t_hi = r_hi.run()
        diffs.append((t_hi - t_lo) / (hi_reps - 1.0))
    diffs.sort()
    med = diffs[len(diffs) // 2]
    print(f"bench_paired: per-body diffs(us) = "
          f"{[f'{d*1e6:.0f}' for d in diffs]} -> median {med*1e6:.0f}us")
    return med * 1e9


class _BenchRunner:
    def __init__(self, inputs, reps):
        import jax
        from jax.experimental.shard_map import shard_map
        from jax.sharding import Mesh, NamedSharding, PartitionSpec
        from concourse import bass2jax, mybir as _mybir

        bass2jax.install_neuronx_cc_hook()
        nc = _get_program(reps)
        in_maps = _make_in_maps(**inputs)
        partition_name = (nc.partition_id_tensor.name
                          if nc.partition_id_tensor else None)
        in_names, out_names, out_avals, zero_outs = [], [], [], []
        for alloc in nc.m.functions[0].allocations:
            if not isinstance(alloc, _mybir.MemoryLocationSet):
                continue
            name = alloc.memorylocations[0].name
            if alloc.kind == "ExternalInput":
                if name != partition_name:
                    in_names.append(name)
            elif alloc.kind == "ExternalOutput":
                shape = tuple(alloc.tensor_shape)
                dtype = _mybir.dt.np(alloc.dtype)
                out_names.append(name)
                out_avals.append(jax.core.ShapedArray(shape, dtype))
                zero_outs.append(np.zeros(shape, dtype))
        n_params = len(in_names)
        all_in = list(in_names) + list(out_names)
        if partition_name is not None:
            all_in.append(partition_name)

        def _body(*args):
            operands = list(args)
            if partition_name is not None:
                operands.append(bass2jax.partition_id_tensor())
            return tuple(bass2jax._bass_exec_p.bind(
                *operands, out_avals=tuple(out_avals), in_names=tuple(all_in),
                out_names=tuple(out_names), lowering_input_output_aliases=(),
                sim_require_finite=True, sim_require_nnan=True, nc=nc))

        donate = tuple(range(n_params, n_params + len(out_avals)))
        devices = jax.devices()[:8]
        mesh = Mesh(np.asarray(devices), ("core",))
        specs = (PartitionSpec("core"),)
        self._sharded = jax.jit(
            shard_map(_body, mesh=mesh,
                      in_specs=specs * (n_params + len(out_avals)),
                      out_specs=specs * len(out_avals), check_rep=False),
            donate_argnums=donate, keep_unused=True)
        sh = NamedSharding(mesh, PartitionSpec("core"))
        self._dev_in = [jax.device_put(
            np.concatenate([np.asarray(in_maps[c][nm]) for c in range(8)],
                           axis=0), sh)
            for nm in in_names]
        self._zero_outs = zero_outs
        self._sh = sh
        self._jax = jax

    def run(self):
        import time
        jax = self._jax
        zs = [jax.device_put(
            np.zeros((8 * z.shape[0], *z.shape[1:]), z.dtype), self._sh)
            for z in self._zero_outs]
        jax.block_until_ready(zs)
        t0 = time.perf_counter()
        out = self._sharded(*self._dev_in, *zs)
        jax.block_until_ready(out)
        return time.perf_counter() - t0

